# revision 1
# baseline (speedup 1.0000x reference)
"""LocalConv2D (3x3, width split into 4 weight blocks, 4-bit fake-quant weights)
on 8 Trainium2 NeuronCores.

Strategy
--------
Data-parallel over batch: 32 images -> 4 per core. Per core, images are
processed in pairs: image A lives in SBUF partitions 0-63 (its 64 channels),
image B in partitions 64-127. The 3x3 conv is 9 shifted K=64 matmuls
accumulated in PSUM; A's matmuls run in PE row-group 0 and B's in row-group
64 (tile_position auto-derived from base partitions), so the two K=64
streams occupy both halves of the 128x128 array concurrently -> full PE
utilization without duplicating input data.

Weights are fake-quantized per-tensor to 4 bits: q = round(w/s)*s with
s = max|w|/7. round(w/s) is a small integer in [-7,7], exactly representable
in bf16, so the matmul runs on exact integer weights and the scale s is
folded into the eviction (ScalarE activation: out = psum*s + bias).
Inputs are pre-padded (H,W +1 each side) and cast to bf16 on host.
"""

import numpy as np

KSIZE = 3
SW = 4
KBITS = 4
N, C, H, W, F = 32, 64, 56, 56, 128
HP, WP = H + 2, W + 2          # padded 58x58
N_CORES = 8
IMGS_PER_CORE = N // N_CORES   # 4
WB = W // SW                   # 14
HH = H // 2                    # 28 rows per h-half tile (28*14 = 392 <= 512)

_COMPILED = {}


def _install_drain_patch():
    """The walrus build here rejects instructions carrying >2 sync waits
    ('Too many sync wait commands'). Two fixes, both relying on engines
    executing their own stream in order:

    1. _add_instruction: any scheduled instruction with >2 waits gets
       same-engine NoOps inserted before it, each carrying <=2 of the waits.
    2. The Tile tail drain gets one wait per outstanding logical proc; emit
       one SP nop per proc, then strip the duplicated waits off the drain.
    """
    import re
    import bass_rust
    from concourse.vector_clock import ScopedClock
    import concourse.tile as tile
    import concourse.mybir as mybir

    if getattr(tile.TileContext, "_drain_patch_installed", False):
        return

    MAXW = 1       # this walrus build fits exactly 1 sync wait per instruction
    NOP_MAXW = 1
    _orig_add = tile.TileContext._add_instruction

    def _add_split(self, inst):
        si = getattr(inst, "sync_info", None)
        if si is not None and si.on_wait and len(si.on_wait) > MAXW:
            waits = list(si.on_wait)
            while len(waits) > MAXW:
                chunk, waits = waits[:NOP_MAXW], waits[NOP_MAXW:]
                nop = mybir.InstNoOp(
                    name=self.nc.get_next_instruction_name(), ins=[], outs=[]
                )
                nop.engine = inst.engine
                nop.sync_info = bass_rust.SyncInfo(on_wait=chunk, on_update=[])
                _orig_add(self, nop)
            si.on_wait = waits
        return _orig_add(self, inst)

    tile.TileContext._add_instruction = _add_split

    _orig = tile.TileContext._drain_and_barrier

    def _split(self, tick_clock, wait_clock):
        gc = tick_clock.global_clock
        m = re.match(r"VectorClock\(\[(.*)\]\)", repr(gc))
        vals = [int(v) for v in m.group(1).split(",")] if m.group(1).strip() else []
        covered = set()
        for i, v in enumerate(vals):
            if v == 0:
                continue
            part = [0] * len(vals)
            part[i] = v
            nop = self.nc.sync.nop()
            wait_clock.add_sem_waits(
                nop.ins, ScopedClock({None: bass_rust.VectorClock(part)})
            )
            si = nop.ins.sync_info
            for w in (si.on_wait if si else []) or []:
                covered.add((w.ant_name, w.wait_value))
        holder = []
        orig_drain = self.nc.sync.drain

        def capture(*a, **k):
            inst = orig_drain(*a, **k)
            holder.append(inst)
            return inst

        self.nc.sync.drain = capture
        try:
            r = _orig(self, tick_clock, wait_clock)
        finally:
            self.nc.sync.drain = orig_drain
        if holder:
            inst = holder[0].ins if hasattr(holder[0], "ins") else holder[0]
            si = inst.sync_info
            if si and si.on_wait:
                si.on_wait = [
                    w for w in si.on_wait
                    if (w.ant_name, w.wait_value) not in covered
                ]
        return r

    tile.TileContext._drain_and_barrier = _split
    tile.TileContext._drain_patch_installed = True


def _build_program():
    import concourse.bass as bass
    import concourse.mybir as mybir
    import concourse.tile as tile

    _install_drain_patch()

    f32 = mybir.dt.float32
    bf16 = mybir.dt.bfloat16

    nc = bass.Bass(target_bir_lowering=False, debug=False)
    # per-core inputs
    xin = nc.declare_dram_parameter(
        "xin", [IMGS_PER_CORE // 2, 128, HP * WP], bf16, isOutput=False
    )
    wts = nc.declare_dram_parameter("wts", [128, SW * 9 * F], bf16, isOutput=False)
    bias_p = nc.declare_dram_parameter("bias", [128, 1], f32, isOutput=False)
    scl_p = nc.declare_dram_parameter("scl", [128, 1], f32, isOutput=False)
    y = nc.declare_dram_parameter(
        "y", [IMGS_PER_CORE, 128, H * W], f32, isOutput=True
    )

    with tile.TileContext(nc) as tc:
        with (
            tc.tile_pool(name="consts", bufs=1) as consts,
            tc.tile_pool(name="strips", bufs=2) as strips,
            tc.tile_pool(name="stage", bufs=2) as stage,
            tc.tile_pool(name="psum", bufs=2, space="PSUM") as psum,
        ):
            wt = consts.tile([128, SW * 9 * F], bf16)
            nc.sync.dma_start(wt[:], wts[:])
            bias_t = consts.tile([128, 1], f32)
            nc.sync.dma_start(bias_t[:], bias_p[:])
            scl_t = consts.tile([128, 1], f32)
            nc.sync.dma_start(scl_t[:], scl_p[:])

            for q in range(IMGS_PER_CORE // 2):
                sq = strips.tile([128, HP * WP], bf16, tag="strip")
                nc.sync.dma_start(sq[:], xin[q])
                sqv = sq.rearrange("p (h w) -> p h w", w=WP)

                st_a = stage.tile([128, H * W], f32, tag="stage_a")
                st_b = stage.tile([128, H * W], f32, tag="stage_b")
                st_av = st_a.rearrange("p (h w) -> p h w", w=W)
                st_bv = st_b.rearrange("p (h w) -> p h w", w=W)

                for b in range(SW):
                    for hh in range(2):
                        ps_a = psum.tile([128, HH * WB], f32, tag="ps_a")
                        ps_b = psum.tile([128, HH * WB], f32, tag="ps_b")
                        h0 = hh * HH
                        for t in range(9):
                            i, j = t // 3, t % 3
                            first, last = t == 0, t == 8
                            wcol = (b * 9 + t) * F
                            rhs_a = sqv[0:64, h0 + i:h0 + i + HH,
                                        b * WB + j:b * WB + j + WB]
                            rhs_b = sqv[64:128, h0 + i:h0 + i + HH,
                                        b * WB + j:b * WB + j + WB]
                            nc.tensor.matmul(
                                ps_a[:], wt[0:64, wcol:wcol + F], rhs_a,
                                start=first, stop=last,
                            )
                            nc.tensor.matmul(
                                ps_b[:], wt[64:128, wcol:wcol + F], rhs_b,
                                start=first, stop=last,
                            )
                        # evict with fused quant-scale + bias:
                        # out = psum * s + bias
                        out_a = st_av[:, h0:h0 + HH, b * WB:(b + 1) * WB]
                        out_b = st_bv[:, h0:h0 + HH, b * WB:(b + 1) * WB]
                        ps_av = ps_a.rearrange("p (h w) -> p h w", w=WB)
                        ps_bv = ps_b.rearrange("p (h w) -> p h w", w=WB)
                        nc.scalar.activation(
                            out_a, ps_av,
                            mybir.ActivationFunctionType.Identity,
                            bias=bias_t[:, 0:1], scale=scl_t[:, 0:1],
                        )
                        nc.vector.tensor_scalar(
                            out_b, ps_bv, scl_t[:, 0:1], bias_t[:, 0:1],
                            mybir.AluOpType.mult, mybir.AluOpType.add,
                        )
                nc.sync.dma_start(y[2 * q], st_a[:])
                nc.sync.dma_start(y[2 * q + 1], st_b[:])

    nc.finalize()
    return nc


def _prep_inputs(inputs, kernel, bias):
    import ml_dtypes

    # fake-quant: integer part exact in bf16, scale folded into eviction
    scale = float(np.max(np.abs(kernel)) / 7.0)
    w_int = np.round(kernel[0] / scale).astype(np.float32)  # [SW, 576, F]

    # weight layout: [128 partitions, SW*9*F]; partition p holds channel
    # c = p % 64 (duplicated across halves for the two PE row groups)
    # free index (b*9 + t)*F + f  ->  w_int[b, c*9 + t, f]
    w4 = w_int.reshape(SW, C, 9, F)                    # [b, c, t, f]
    wt_half = np.transpose(w4, (1, 0, 2, 3)).reshape(C, SW * 9 * F)
    wts_host = np.concatenate([wt_half, wt_half], axis=0)  # [128, SW*9*F]
    wts_host = wts_host.astype(ml_dtypes.bfloat16)

    # input: pad spatially, bf16, pack image pairs into 128 partitions
    xp = np.zeros((N, C, HP, WP), np.float32)
    xp[:, :, 1:-1, 1:-1] = inputs
    xp = xp.astype(ml_dtypes.bfloat16).reshape(N, C, HP * WP)

    bias_host = np.ascontiguousarray(bias, np.float32).reshape(128, 1)
    scl_host = np.full((128, 1), scale, np.float32)

    in_maps = []
    for core in range(N_CORES):
        base = core * IMGS_PER_CORE
        xin = np.empty((IMGS_PER_CORE // 2, 128, HP * WP), ml_dtypes.bfloat16)
        for q in range(IMGS_PER_CORE // 2):
            xin[q, 0:64] = xp[base + 2 * q]
            xin[q, 64:128] = xp[base + 2 * q + 1]
        in_maps.append({
            "xin": xin,
            "wts": wts_host,
            "bias": bias_host,
            "scl": scl_host,
        })
    return in_maps


def kernel(inputs, kernel, bias, _trace=False):
    from concourse.bass_utils import run_bass_kernel_spmd

    inputs = np.asarray(inputs)
    kernel = np.asarray(kernel)
    bias = np.asarray(bias)

    if "nc" not in _COMPILED:
        _COMPILED["nc"] = _build_program()
    nc = _COMPILED["nc"]

    in_maps = _prep_inputs(inputs, kernel, bias)
    res = run_bass_kernel_spmd(
        nc, in_maps, list(range(N_CORES)), trace=_trace
    )
    out = np.empty((N, F, H, W), np.float32)
    for core in range(N_CORES):
        base = core * IMGS_PER_CORE
        out[base:base + IMGS_PER_CORE] = (
            res.results[core]["y"].reshape(IMGS_PER_CORE, F, H, W)
        )
    if _trace:
        return out, res
    return out

